# revision 1
# baseline (speedup 1.0000x reference)
import os
from contextlib import ExitStack

import numpy as np

import concourse.bass as bass
import concourse.mybir as mybir
from concourse.bass_utils import run_bass_kernel_spmd

F32 = mybir.dt.float32
AF = mybir.ActivationFunctionType
OP = mybir.AluOpType

T = 4096
ROWS = 128
NCORES = 8
SIGMAS = (2.5, 4.0, 6.0, 9.0, 14.0)
RMAX = 56
XPW = T + 2 * RMAX
WIN = 16
TC = 1024
NCH = T // TC

LAST_EXEC_NS = None


def _gk(sigma):
    R = max(1, int(4.0 * sigma + 0.5))
    R = min(R, max(1, (T - 1) // 2))
    xs = np.arange(-R, R + 1, dtype=np.float32)
    k = np.exp(np.float32(-0.5) * (xs / np.float32(sigma)) ** 2).astype(np.float32)
    k = k / (k.sum() + np.float32(1e-12))
    return R, [float(v) for v in k]


class Ser:
    """Serial cross-engine scheduler: buffers (engine, emit_fn) in program
    order, then replays per-engine with standalone wait_ge for cross-engine
    deps and then_inc on the last op before each engine switch."""

    def __init__(self):
        self.ops = []  # [eng, fn, is_dma]

    def add(self, eng, fn, dma=False):
        self.ops.append([eng, fn, dma])

    def emit(self, nc, sems):
        # pass 1: decide incs (on last op before engine switch) and waits
        n = len(self.ops)
        incs = [None] * n     # (sem_name, amt)
        waits = [[] for _ in range(n)]  # list of (sem_name, value)
        cnt = {e: 0 for e in sems}
        observed = {e: {o: 0 for o in sems} for e in sems}
        for i, (eng, fn, dma) in enumerate(self.ops):
            prev = self.ops[i - 1] if i > 0 else None
            if prev is not None and prev[0] != eng and incs[i - 1] is None:
                # close out previous engine: inc its sem on its last op
                incs[i - 1] = (prev[0], 1)
                cnt[prev[0]] += 1
            # waits for this op: observe all other engines' current counts
            for o in sems:
                if o != eng and observed[eng][o] < cnt[o]:
                    waits[i].append((o, cnt[o]))
                    observed[eng][o] = cnt[o]
            if dma:
                # every DMA must carry sync info
                incs[i] = (eng, 16)
                cnt[eng] += 16
        if incs[-1] is None:
            last_eng, _, last_dma = self.ops[-1]
            incs[-1] = (last_eng, 1)
            cnt[last_eng] += 1

        # pass 2: replay per engine
        per = {e: [] for e in sems}
        for i, (eng, fn, dma) in enumerate(self.ops):
            per[eng].append((i, fn, waits[i], incs[i]))
        return per, cnt


def _build(W1, b1, W2, b2):
    nc = bass.Bass()
    xpad_d = nc.dram_tensor("xpad", [ROWS, XPW], F32, kind="ExternalInput")
    out_d = nc.dram_tensor("out", [ROWS, T], F32, kind="ExternalOutput")

    w2p = (np.asarray(W2, np.float64) / 0.7)
    b2p = (np.asarray(b2, np.float64) / 0.7)
    W1 = np.asarray(W1, np.float64)
    b1 = np.asarray(b1, np.float64)

    # const APs for activation biases (mirrors Bass.__init__ preamble)
    def reg_const(val):
        key = (F32, val)
        if key not in nc.const_aps.aps:
            t = nc.alloc_sbuf_tensor(
                f"const-f32-u{len(nc.const_aps.aps)}", [128, 1], F32)
            nc.gpsimd.memset(t.ap(), val)
            nc.const_aps.aps[key] = t.ap()

    for j in range(32):
        reg_const(float(b1[j]))
    nc.all_engine_barrier()

    with ExitStack() as ctx:
        XP1 = T + WIN - 1
        sb = lambda name, w: ctx.enter_context(  # noqa: E731
            nc.sbuf_tensor(name, [ROWS, w], F32))

        xpad = sb("xpad_sb", XPW)
        z = sb("z", T)
        lv = sb("lv", T)
        ls = [sb(f"l{s}", TC) for s in range(5)]
        h0 = sb("h0", TC)
        h1 = sb("h1", TC)
        mx = sb("mx", TC)
        tmp = sb("tmp", TC)
        sctx = ExitStack()
        ssb = lambda name, w: sctx.enter_context(  # noqa: E731
            nc.sbuf_tensor(name, [ROWS, w], F32))
        xps = ssb("xps", XP1)
        xps2 = ssb("xps2", XP1)
        A = ssb("A", XP1)
        B = ssb("B", XP1)
        inv15 = ssb("inv15", WIN - 1)

        xv = xpad[:, RMAX:RMAX + T]

        S = Ser()
        V, A_, G = "v", "a", "g"

        # ---- input DMA ----
        S.add(G, lambda g: g.dma_start(xpad[:], xpad_d[:]), dma=True)

        # ---- stats ----
        for t in range(WIN - 1):
            val = float(1.0 / (t + 1 + 1e-12))
            S.add(V, lambda v, t=t, val=val: v.memset(inv15[:, t:t + 1], val))
        S.add(V, lambda v: v.tensor_copy(xps[:, WIN - 1:], xv))
        S.add(V, lambda v: v.tensor_copy(
            xps[:, 0:WIN - 1], xv[:, 0:1].to_broadcast((ROWS, WIN - 1))))
        S.add(A_, lambda a: a.activation(xps2[:], xps[:], AF.Square))

        def win16(src, dst):
            S.add(V, lambda v: v.tensor_add(A[:, 0:4110], src[:, 0:4110], src[:, 1:4111]))
            S.add(V, lambda v: v.tensor_add(B[:, 0:4108], A[:, 0:4108], A[:, 2:4110]))
            S.add(V, lambda v: v.tensor_add(A[:, 0:4104], B[:, 0:4104], B[:, 4:4108]))
            S.add(V, lambda v: v.tensor_add(dst, A[:, 0:T], A[:, 8:8 + T]))

        win16(xps, z[:])     # Sx  -> z
        win16(xps2, lv[:])   # Sx2 -> lv

        mean = A[:, 0:T]
        mean2 = B[:, 0:T]
        s16 = float(1.0 / (16.0 + 1e-12))
        S.add(V, lambda v: v.tensor_scalar_mul(mean, z[:], s16))
        S.add(V, lambda v: v.tensor_scalar_mul(mean2, lv[:], s16))
        S.add(V, lambda v: v.tensor_mul(mean[:, 0:WIN - 1], z[:, 0:WIN - 1], inv15[:]))
        S.add(V, lambda v: v.tensor_mul(mean2[:, 0:WIN - 1], lv[:, 0:WIN - 1], inv15[:]))

        msq = xps[:, 0:T]
        var = xps2[:, 0:T]
        S.add(V, lambda v: v.tensor_mul(msq, mean, mean))
        S.add(V, lambda v: v.tensor_sub(var, mean2, msq))
        S.add(V, lambda v: v.tensor_scalar_max(var, var, 0.0))
        S.add(V, lambda v: v.tensor_scalar_add(var, var, 1e-6))

        sd = xps[:, 0:T]
        rsd = lv[:]
        S.add(A_, lambda a: a.activation(sd, var, AF.Sqrt, bias=0.0))
        S.add(V, lambda v: v.reciprocal(rsd, sd))
        S.add(V, lambda v: v.tensor_sub(z[:], xv, mean))
        S.add(V, lambda v: v.tensor_mul(z[:], z[:], rsd))
        S.add(A_, lambda a: a.activation(lv[:], var, AF.Ln, bias=0.0))

        # ---- gaussian convs on DVE ----
        sctx.close()  # free stats scratch; convs run after stats (serial)
        Ys = [sb(f"Y{s}", T) for s in range(5)]
        for si, sig in enumerate(SIGMAS):
            R, k = _gk(sig)
            base = RMAX - R
            S.add(V, lambda v, si=si, base=base, k0=k[0]:
                  v.tensor_scalar_mul(Ys[si][:], xpad[:, base:base + T], k0))
            for j in range(1, 2 * R + 1):
                S.add(V, lambda v, si=si, o=base + j, kj=k[j]:
                      v.scalar_tensor_tensor(Ys[si][:], xpad[:, o:o + T], kj,
                                             Ys[si][:], OP.mult, OP.add))

        # ---- gating MLP + softmax + mix ----
        nch_run = 1 if os.environ.get("DBG_STAGE") == "2" else NCH
        for cidx in range(nch_run):
            c0 = cidx * TC
            zc = z[:, c0:c0 + TC]
            lvc = lv[:, c0:c0 + TC]
            for s in range(5):
                S.add(V, lambda v, s=s: v.memset(ls[s][:], float(b2p[s])))
            for j in range(32):
                a = float(W1[j, 0]); b = float(W1[j, 1]); cj = float(b1[j])
                h = (h0 if j % 2 == 0 else h1)
                if a == 0.0 and b == 0.0:
                    S.add(A_, lambda e, h=h, cj=cj, zc=zc:
                          e.activation(h[:], zc, AF.Gelu, bias=cj, scale=0.0))
                elif abs(a) >= abs(b):
                    S.add(V, lambda v, h=h, r=b / a, zc=zc, lvc=lvc:
                          v.scalar_tensor_tensor(h[:], lvc, r, zc, OP.mult, OP.add))
                    S.add(A_, lambda e, h=h, cj=cj, sc=a:
                          e.activation(h[:], h[:], AF.Gelu, bias=cj, scale=sc))
                else:
                    S.add(V, lambda v, h=h, r=a / b, zc=zc, lvc=lvc:
                          v.scalar_tensor_tensor(h[:], zc, r, lvc, OP.mult, OP.add))
                    S.add(A_, lambda e, h=h, cj=cj, sc=b:
                          e.activation(h[:], h[:], AF.Gelu, bias=cj, scale=sc))
                for s in range(5):
                    S.add(V, lambda v, s=s, h=h, w=float(w2p[s, j]):
                          v.scalar_tensor_tensor(ls[s][:], h[:], w, ls[s][:],
                                                 OP.mult, OP.add))

            S.add(V, lambda v: v.tensor_tensor(mx[:], ls[0][:], ls[1][:], OP.max))
            for s in (2, 3, 4):
                S.add(V, lambda v, s=s: v.tensor_tensor(mx[:], mx[:], ls[s][:], OP.max))
            for s in range(5):
                S.add(V, lambda v, s=s: v.tensor_sub(ls[s][:], ls[s][:], mx[:]))
            for s in range(5):
                S.add(A_, lambda a, s=s: a.activation(ls[s][:], ls[s][:], AF.Exp))
            S.add(V, lambda v: v.tensor_add(mx[:], ls[0][:], ls[1][:]))
            for s in (2, 3, 4):
                S.add(V, lambda v, s=s: v.tensor_add(mx[:], mx[:], ls[s][:]))
            S.add(V, lambda v: v.reciprocal(mx[:], mx[:]))

            S.add(V, lambda v, c0=c0: v.tensor_mul(h0[:], ls[0][:], Ys[0][:, c0:c0 + TC]))
            for s in range(1, 5):
                S.add(V, lambda v, s=s, c0=c0:
                      v.tensor_mul(tmp[:], ls[s][:], Ys[s][:, c0:c0 + TC]))
                S.add(V, lambda v: v.tensor_add(h0[:], h0[:], tmp[:]))
            S.add(V, lambda v: v.tensor_mul(h0[:], h0[:], mx[:]))
            S.add(G, lambda g, c0=c0: g.dma_start(out_d[:, c0:c0 + TC], h0[:]),
                  dma=True)

        # ---- debug probe: dump intermediates into output quarters ----
        if os.environ.get("DBG_STAGE") == "1":
            Q = T // 4
            S.add(G, lambda g: g.dma_start(out_d[:, 0:Q], z[:, 0:Q]), dma=True)
            S.add(G, lambda g: g.dma_start(out_d[:, Q:2*Q], lv[:, Q:2*Q]), dma=True)
            S.add(G, lambda g: g.dma_start(out_d[:, 2*Q:3*Q], Ys[0][:, 2*Q:3*Q]), dma=True)
            S.add(G, lambda g: g.dma_start(out_d[:, 3*Q:4*Q], Ys[4][:, 3*Q:4*Q]), dma=True)
        elif os.environ.get("DBG_STAGE") == "2":
            S.add(G, lambda g: g.dma_start(out_d[:, 0:TC], ls[0][:]), dma=True)
            S.add(G, lambda g: g.dma_start(out_d[:, TC:2*TC], ls[1][:]), dma=True)
            S.add(G, lambda g: g.dma_start(out_d[:, 2*TC:3*TC], ls[2][:]), dma=True)
            S.add(G, lambda g: g.dma_start(out_d[:, 3*TC:4*TC], mx[:]), dma=True)

        # ---- emit with semaphores ----
        with nc.semaphore("v_sem") as v_sem, \
             nc.semaphore("a_sem") as a_sem, \
             nc.semaphore("g_sem") as g_sem, \
             nc.Block() as block:
            semmap = {"v": v_sem, "a": a_sem, "g": g_sem}
            per, cnt = S.emit(nc, semmap)

            def replay(eng_obj, eng_name):
                for i, fn, ws, inc in per[eng_name]:
                    for (o, val) in ws:
                        eng_obj.wait_ge(semmap[o], val)
                    inst = fn(eng_obj)
                    if inc is not None:
                        inst.then_inc(semmap[inc[0]], inc[1])

            @block.vector
            def _(vector):
                replay(nc.vector, "v")

            @block.scalar
            def _(scalar):
                replay(nc.scalar, "a")

            @block.gpsimd
            def _(gpsimd):
                replay(nc.gpsimd, "g")

    return nc


def kernel(x, W1, b1, W2, b2):
    global LAST_EXEC_NS
    x = np.asarray(x, np.float32)
    B, T_, C = x.shape
    xr = np.ascontiguousarray(np.transpose(x, (0, 2, 1))).reshape(B * C, T_)
    xp = np.pad(xr, ((0, 0), (RMAX, RMAX)), mode="reflect").astype(np.float32)

    nc = _build(W1, b1, W2, b2)
    in_maps = [
        {"xpad": np.ascontiguousarray(xp[i * ROWS:(i + 1) * ROWS])}
        for i in range(NCORES)
    ]
    trace = bool(os.environ.get("KBENCH_TRACE"))
    res = run_bass_kernel_spmd(nc, in_maps, core_ids=list(range(NCORES)),
                               trace=trace)
    LAST_EXEC_NS = getattr(res, "exec_time_ns", None)
    outs = np.concatenate([np.asarray(res.results[i]["out"])
                           for i in range(NCORES)], axis=0)
    return np.ascontiguousarray(
        outs.reshape(B, C, T_).transpose(0, 2, 1)).astype(np.float32)



# revision 2
# speedup vs baseline: 2.2148x; 2.2148x over previous
import os
from contextlib import ExitStack

import numpy as np

import jax

jax.config.update("jax_compilation_cache_dir",
                  os.path.expanduser("~/.cache/jax_bass_cache"))
jax.config.update("jax_persistent_cache_min_compile_time_secs", 0.0)
jax.config.update("jax_persistent_cache_min_entry_size_bytes", -1)

import concourse.bass as bass
import concourse.mybir as mybir
from concourse.bass_utils import run_bass_kernel_spmd

F32 = mybir.dt.float32
F16 = mybir.dt.float16
AF = mybir.ActivationFunctionType
OP = mybir.AluOpType

T = 4096
ROWS = 128
NCORES = 8
NBLK = 32           # data blocks of 128 along time
NBLKP = NBLK + 2    # plus one reflect-pad block each side
PADW = NBLKP * 128  # 4352
HW = 2048           # half width (16 blocks)
SIGMAS = (2.5, 4.0, 6.0, 9.0, 14.0)
ZCLAMP = 200.0
NKM = 19            # stationary mats: ident, K0, K1, K0f, 5 sigmas x 3

LAST_EXEC_NS = None


def _gk(sigma):
    R = max(1, int(4.0 * sigma + 0.5))
    R = min(R, max(1, (T - 1) // 2))
    xs = np.arange(-R, R + 1, dtype=np.float32)
    k = np.exp(np.float32(-0.5) * (xs / np.float32(sigma)) ** 2).astype(np.float32)
    return R, k / (k.sum() + np.float32(1e-12))


def _stationaries():
    """[128, NKM*128] fp16. Layout: 0 identity, 1 K0box, 2 K1box, 3 K0fbox,
    4+3s+g gaussian sigma s shift g. Matmul computes out[u,r] = sum_p M[p,u]X[p,r]."""
    p = np.arange(128)[:, None]
    u = np.arange(128)[None, :]
    mats = np.zeros((NKM, 128, 128), np.float32)
    mats[0] = np.eye(128, dtype=np.float32)
    mats[1] = ((p >= u - 15) & (p <= u)).astype(np.float32)          # K0 causal box
    mats[2] = ((p - 128 >= u - 15) & (p - 128 <= u)).astype(np.float32)  # K1 prev blk
    mats[3] = mats[1].copy()                                          # K0f: + replicate
    for uu in range(15):
        mats[3][0, uu] += 15 - uu
    for si, sig in enumerate(SIGMAS):
        R, k = _gk(sig)
        for g in range(3):
            j = (g - 1) * 128 + p - u + R
            m = (j >= 0) & (j <= 2 * R)
            mats[4 + 3 * si + g][m] = k[np.clip(j, 0, 2 * R)][m]
    km = np.concatenate([mats[i] for i in range(NKM)], axis=1)
    return np.ascontiguousarray(km.astype(np.float16))


_KM = _stationaries()
_CE = (1.0 / np.minimum(np.arange(1, 129, dtype=np.float32), 16.0)).reshape(128, 1)


class _Op:
    __slots__ = ("eng", "fn", "inc", "cnt", "deps", "dma")

    def __init__(self, eng, fn, inc, cnt, deps, dma):
        self.eng, self.fn, self.inc, self.cnt = eng, fn, inc, cnt
        self.deps, self.dma = deps, dma


class Sched:
    """Per-engine in-order queues with explicit cross-engine deps, emitted as
    one semaphore per engine (then_inc after every op, wait_ge before ops with
    unseen dependency counts)."""

    ENGS = ("p", "a", "v", "g", "s")

    def __init__(self):
        self.ops = []
        self.cnt = {e: 0 for e in self.ENGS}

    def add(self, eng, fn, deps=(), dma=False):
        inc = 16 if dma else 1
        self.cnt[eng] += inc
        op = _Op(eng, fn, inc, self.cnt[eng], tuple(d for d in deps if d is not None),
                 dma)
        self.ops.append(op)
        return op

    def emit(self, nc):
        per = {e: [] for e in self.ENGS}
        observed = {e: {o: 0 for o in self.ENGS} for e in self.ENGS}
        for op in self.ops:
            ws = []
            for dep in op.deps:
                if dep.eng == op.eng and not dep.dma:
                    continue  # same-engine program order
                if observed[op.eng][dep.eng] < dep.cnt:
                    ws.append((dep.eng, dep.cnt))
                    observed[op.eng][dep.eng] = dep.cnt
            per[op.eng].append((op, ws))

        with ExitStack() as sctx:
            sems = {e: sctx.enter_context(nc.semaphore(f"sem_{e}"))
                    for e in self.ENGS}
            with nc.Block() as block:
                def replay(engobj, ename):
                    for op, ws in per[ename]:
                        for (o, val) in ws:
                            engobj.wait_ge(sems[o], val)
                        op.fn(engobj).then_inc(sems[ename], op.inc)
                    if ename == "s" and self.cnt["s"]:
                        engobj.wait_ge(sems["s"], self.cnt["s"])

                @block.tensor
                def _(e):
                    replay(nc.tensor, "p")

                @block.scalar
                def _(e):
                    replay(nc.scalar, "a")

                @block.vector
                def _(e):
                    replay(nc.vector, "v")

                @block.gpsimd
                def _(e):
                    replay(nc.gpsimd, "g")

                @block.sync
                def _(e):
                    replay(nc.sync, "s")


def _build(W1, b1, W2, b2, detect_races=True):
    nc = bass.Bass(detect_race_conditions=detect_races)
    xr_d = nc.dram_tensor("xr", [ROWS, PADW], F16, kind="ExternalInput")
    km_d = nc.dram_tensor("km", [ROWS, NKM * 128], F16, kind="ExternalInput")
    ce_d = nc.dram_tensor("ce", [ROWS, 1], F32, kind="ExternalInput")
    out_d = nc.dram_tensor("out", [ROWS, T], F16, kind="ExternalOutput")

    W1 = np.asarray(W1, np.float64)
    b1 = np.asarray(b1, np.float64)
    w2p = np.asarray(W2, np.float64) / 0.7
    b2p = np.asarray(b2, np.float64) / 0.7

    # Pre-register activation bias consts (activation() converts float biases
    # to const APs, which must be materialized before the Block bodies).
    def reg_const(val):
        key = (F32, float(val))
        if key not in nc.const_aps.aps:
            t = nc.alloc_sbuf_tensor(f"c-{len(nc.const_aps.aps)}", [128, 1], F32)
            nc.gpsimd.memset(t.ap(), float(val))
            nc.const_aps.aps[key] = t.ap()

    for v in [0.0, 1e-6] + [float(x) for x in b1]:
        reg_const(v)
    nc.all_engine_barrier()

    with nc.allow_low_precision("fp16 pipeline by design"), ExitStack() as ctx:
        sb = lambda name, w, dt=F16: ctx.enter_context(  # noqa: E731
            nc.sbuf_tensor(name, [ROWS, w], dt))

        xrow = sb("xrow", PADW)
        km = sb("km_sb", NKM * 128)
        ce = sb("ce_sb", 1, F32)
        Xt = sb("Xt", PADW)
        xsq = sb("xsq", T)
        z = sb("z", T)
        lv = sb("lv", T)
        Ys = [sb(f"Y{s}", T) for s in range(5)]
        las = [sb(f"la{s}", T) for s in range(5)]
        hb = [sb("hb0", T), sb("hb1", T)]
        sm1 = sb("sm1", HW, F32)
        sm2 = sb("sm2", HW, F32)
        sm3 = sb("sm3", HW, F32)
        acc = sb("acc", T)
        outr = sb("outr", T)

        ident = km[:, 0:128]

        def kmat(i):
            return km[:, i * 128:(i + 1) * 128]

        S = Sched()

        d_x = S.add("s", lambda e: e.dma_start(xrow[:], xr_d[:]), dma=True)
        d_k = S.add("s", lambda e: e.dma_start(km[:], km_d[:]), dma=True)
        d_c = S.add("s", lambda e: e.dma_start(ce[:], ce_d[:]), dma=True)

        # ---- phase 1: transpose input into time-major blocks ----
        cps = []
        with ExitStack() as pctx:
            pts = [pctx.enter_context(nc.psum_tensor(f"pt{i}", [128, 128], F16))
                   for i in range(4)]
            for i in range(NBLKP):
                deps = [d_x, d_k] + ([cps[i - 4]] if i >= 4 else [])
                tr = S.add("p", lambda e, i=i, pt=pts[i % 4]: e.transpose(
                    pt[:], xrow[:, i * 128:(i + 1) * 128], ident), deps)
                cps.append(S.add("a", lambda e, i=i, pt=pts[i % 4]: e.activation(
                    Xt[:, i * 128:(i + 1) * 128], pt[:], AF.Copy), [tr]))
        xsq_op = S.add("a", lambda e: e.activation(
            xsq[:], Xt[:, 128:128 + T], AF.Square))

        # ---- phase 2: causal window sums via PE + stats math ----
        with ExitStack() as pctx:
            ps1 = pctx.enter_context(nc.psum_tensor("ps1", [128, HW], F32))
            ps2 = pctx.enter_context(nc.psum_tensor("ps2", [128, HW], F32))
            ps_readers = {0: [cps[-1]], 1: [cps[-1], xsq_op]}
            zlv_ops = []
            sm_free = []   # ops that must finish before sm1/sm2/sm3 are reused

            def box_mms(ps, k0src, k1src, special, deps):
                """Causal box sums into ps[:, 0:HW], chunked per PSUM bank.
                k0src(lo, hi) / k1src(lo, hi) give moving APs for the chunk;
                special: (kidx, ap) overrides chunk [0:128] with one matmul."""
                ops = []
                lo0 = 0
                if special is not None:
                    kidx, ap = special
                    ops.append(S.add("p", lambda e, kidx=kidx, ap=ap: e.matmul(
                        ps[:, 0:128], kmat(kidx), ap, start=True, stop=True),
                        deps))
                    deps = ()
                    lo0 = 128
                for ci in range(4):
                    lo, hi = max(512 * ci, lo0), 512 * (ci + 1)
                    ops.append(S.add("p", lambda e, lo=lo, hi=hi: e.matmul(
                        ps[:, lo:hi], kmat(1), k0src(lo, hi),
                        start=True, stop=False), deps))
                    deps = ()
                    ops.append(S.add("p", lambda e, lo=lo, hi=hi: e.matmul(
                        ps[:, lo:hi], kmat(2), k1src(lo, hi),
                        start=False, stop=True)))
                return ops

            for h in range(2):
                c0 = h * HW
                if h == 0:
                    mS = box_mms(ps1,
                                 lambda lo, hi: Xt[:, 128 + lo:128 + hi],
                                 lambda lo, hi: Xt[:, lo:hi],
                                 (3, Xt[:, 128:256]), ps_readers[0])
                    mQ = box_mms(ps2,
                                 lambda lo, hi: xsq[:, lo:hi],
                                 lambda lo, hi: xsq[:, lo - 128:hi - 128],
                                 (3, xsq[:, 0:128]), ps_readers[1])
                else:
                    mS = box_mms(ps1,
                                 lambda lo, hi: Xt[:, 128 + HW + lo:128 + HW + hi],
                                 lambda lo, hi: Xt[:, HW + lo:HW + hi],
                                 None, ps_readers[0])
                    mQ = box_mms(ps2,
                                 lambda lo, hi: xsq[:, HW + lo:HW + hi],
                                 lambda lo, hi: xsq[:, HW - 128 + lo:HW - 128 + hi],
                                 None, ps_readers[1])
                # mean / mean2 (PSUM f32 -> SBUF f32, per-position 1/eff)
                if h == 0:
                    am1a = S.add("a", lambda e: e.activation(
                        sm1[:, 0:128], ps1[:, 0:128], AF.Copy,
                        scale=ce[:, 0:1]), [mS[-1], d_c])
                    am1 = S.add("a", lambda e: e.activation(
                        sm1[:, 128:HW], ps1[:, 128:HW], AF.Copy,
                        scale=1.0 / 16.0), [mS[-1]])
                    am2a = S.add("a", lambda e: e.activation(
                        sm2[:, 0:128], ps2[:, 0:128], AF.Copy,
                        scale=ce[:, 0:1]), [mQ[-1], d_c])
                    am2 = S.add("a", lambda e: e.activation(
                        sm2[:, 128:HW], ps2[:, 128:HW], AF.Copy,
                        scale=1.0 / 16.0), [mQ[-1]])
                    mean_ops = [am1a, am1]
                    mean2_ops = [am2a, am2]
                else:
                    am1 = S.add("a", lambda e: e.activation(
                        sm1[:], ps1[:], AF.Copy, scale=1.0 / 16.0),
                        [mS[-1]] + sm_free)
                    am2 = S.add("a", lambda e: e.activation(
                        sm2[:], ps2[:], AF.Copy, scale=1.0 / 16.0),
                        [mQ[-1]] + sm_free)
                    mean_ops = [am1]
                    mean2_ops = [am2]
                ps_readers = {0: mean_ops, 1: mean2_ops}

                v1 = S.add("v", lambda e: e.tensor_mul(sm3[:], sm1[:], sm1[:]),
                           mean_ops)
                v2 = S.add("v", lambda e: e.tensor_sub(sm2[:], sm2[:], sm3[:]),
                           mean2_ops)
                v3 = S.add("v", lambda e: e.tensor_scalar_max(sm2[:], sm2[:], 0.0))
                a3 = S.add("a", lambda e: e.activation(
                    sm3[:], sm2[:], AF.Sqrt, bias=1e-6), [v3])
                a4 = S.add("a", lambda e, c0=c0: e.activation(
                    lv[:, c0:c0 + HW], sm2[:], AF.Ln, bias=1e-6), [v3])
                v4 = S.add("v", lambda e: e.reciprocal(sm3[:], sm3[:]), [a3])
                v5 = S.add("v", lambda e, c0=c0: e.tensor_sub(
                    sm1[:], Xt[:, 128 + c0:128 + c0 + HW], sm1[:]), [a4])
                v6 = S.add("v", lambda e: e.tensor_mul(sm1[:], sm1[:], sm3[:]))
                v7 = S.add("v", lambda e, c0=c0: e.tensor_scalar(
                    z[:, c0:c0 + HW], sm1[:], ZCLAMP, -ZCLAMP, OP.min, OP.max))
                zlv_ops += [v7, a4]
                sm_free = [v7, v6, a4]
                ps_readers = {0: mean_ops, 1: mean2_ops}

            # ---- phase 3: gaussian convs via PE ----
            pgs = [ps1, ps2]
            g_copy = []
            for idx in range(10):
                si, h = divmod(idx, 2)
                c0 = h * HW
                pg = pgs[idx % 2]
                deps = [g_copy[idx - 2]] if idx >= 2 else list(ps_readers[idx])
                last = None
                for g in range(3):
                    for ci in range(4):
                        lo, hi = 512 * ci, 512 * (ci + 1)
                        last = S.add("p", lambda e, si=si, g=g, pg=pg,
                                     s0=c0 + g * 128 + lo, s1=c0 + g * 128 + hi,
                                     lo=lo, hi=hi: e.matmul(
                                         pg[:, lo:hi], kmat(4 + 3 * si + g),
                                         Xt[:, s0:s1],
                                         start=(g == 0), stop=(g == 2)), deps)
                        deps = ()
                g_copy.append(S.add("a", lambda e, si=si, c0=c0, pg=pg:
                                    e.activation(Ys[si][:, c0:c0 + HW], pg[:],
                                                 AF.Copy), [last]))

        # ---- phase 4: gating MLP (elementwise, DVE + ACT) ----
        gels = []
        for j in range(32):
            a = float(W1[j, 0])
            b = float(W1[j, 1])
            cj = float(b1[j])
            h = hb[j % 2]
            hbfree = [gels[j - 2]] if j >= 2 else []
            if a == 0.0 and b == 0.0:
                gel = S.add("a", lambda e, h=h, cj=cj: e.activation(
                    h[:], z[:], AF.Gelu, bias=cj, scale=0.0), zlv_ops + hbfree)
            else:
                if abs(a) >= abs(b):
                    pre = S.add("v", lambda e, h=h, r=b / a: e.scalar_tensor_tensor(
                        h[:], lv[:], r, z[:], OP.mult, OP.add), zlv_ops + hbfree)
                    sc = a
                else:
                    pre = S.add("v", lambda e, h=h, r=a / b: e.scalar_tensor_tensor(
                        h[:], z[:], r, lv[:], OP.mult, OP.add), zlv_ops + hbfree)
                    sc = b
                gel = S.add("a", lambda e, h=h, cj=cj, sc=sc: e.activation(
                    h[:], h[:], AF.Gelu, bias=cj, scale=sc), [pre])
            gels.append(gel)
            for s in range(5):
                w = float(w2p[s, j])
                if j == 0:
                    S.add("v", lambda e, s=s, h=h, w=w, b0=float(b2p[s]):
                          e.tensor_scalar(las[s][:], h[:], w, b0, OP.mult, OP.add),
                          [gel])
                else:
                    S.add("v", lambda e, s=s, h=h, w=w: e.scalar_tensor_tensor(
                        las[s][:], h[:], w, las[s][:], OP.mult, OP.add), [gel])

        # ---- phase 5: softmax + mix ----
        mx = hb[0]
        den = hb[1]
        S.add("v", lambda e: e.tensor_tensor(mx[:], las[0][:], las[1][:], OP.max),
              [gels[-1]])
        for s in (2, 3, 4):
            S.add("v", lambda e, s=s: e.tensor_tensor(mx[:], mx[:], las[s][:],
                                                      OP.max))
        subs = [S.add("v", lambda e, s=s: e.tensor_sub(las[s][:], las[s][:], mx[:]))
                for s in range(5)]
        exps = [S.add("a", lambda e, s=s: e.activation(las[s][:], las[s][:], AF.Exp),
                      [subs[s]]) for s in range(5)]
        S.add("v", lambda e: e.tensor_add(den[:], las[0][:], las[1][:]),
              [exps[0], exps[1]])
        for s in (2, 3, 4):
            S.add("v", lambda e, s=s: e.tensor_add(den[:], den[:], las[s][:]),
                  [exps[s]])
        S.add("v", lambda e: e.reciprocal(den[:], den[:]))
        S.add("v", lambda e: e.tensor_mul(acc[:], las[0][:], Ys[0][:]))
        tmps = [z, lv]
        for s in range(1, 5):
            t = tmps[(s - 1) % 2]
            S.add("v", lambda e, s=s, t=t: e.tensor_mul(t[:], las[s][:], Ys[s][:]))
            S.add("v", lambda e, t=t: e.tensor_add(acc[:], acc[:], t[:]))
        vfin = S.add("v", lambda e: e.tensor_mul(acc[:], acc[:], den[:]))

        # ---- phase 6: transpose back to row-major and store ----
        with ExitStack() as pctx:
            pts = [pctx.enter_context(nc.psum_tensor(f"pu{i}", [128, 128], F16))
                   for i in range(4)]
            ocp = []
            for bidx in range(NBLK):
                deps = [vfin, g_copy[-1]] + ([ocp[bidx - 4]] if bidx >= 4 else [])
                tr = S.add("p", lambda e, b=bidx, pt=pts[bidx % 4]: e.transpose(
                    pt[:], acc[:, b * 128:(b + 1) * 128], ident), deps)
                ocp.append(S.add("a", lambda e, b=bidx, pt=pts[bidx % 4]:
                                 e.activation(outr[:, b * 128:(b + 1) * 128],
                                              pt[:], AF.Copy), [tr]))
        S.add("s", lambda e: e.dma_start(out_d[:], outr[:]), [ocp[-1]], dma=True)

        S.emit(nc)
    return nc


_PROG_CACHE = {}


def _get_program(W1, b1, W2, b2):
    key = (np.asarray(W1, np.float32).tobytes(), np.asarray(b1, np.float32).tobytes(),
           np.asarray(W2, np.float32).tobytes(), np.asarray(b2, np.float32).tobytes())
    prog = _PROG_CACHE.get(key)
    if prog is None:
        prog = _build(W1, b1, W2, b2)
        _PROG_CACHE.clear()
        _PROG_CACHE[key] = prog
    return prog


def kernel(x, W1, b1, W2, b2):
    global LAST_EXEC_NS
    x = np.asarray(x)
    B, T_, C = x.shape
    xr16 = np.ascontiguousarray(
        x.astype(np.float16).transpose(0, 2, 1)).reshape(B * C, T_)
    xp = np.pad(xr16, ((0, 0), (128, 128)), mode="reflect")

    nc = _get_program(W1, b1, W2, b2)
    in_maps = [
        {"xr": xp[i * ROWS:(i + 1) * ROWS], "km": _KM, "ce": _CE}
        for i in range(NCORES)
    ]
    res = run_bass_kernel_spmd(nc, in_maps, core_ids=list(range(NCORES)),
                               trace=bool(os.environ.get("KBENCH_TRACE")))
    LAST_EXEC_NS = getattr(res, "exec_time_ns", None)
    outs = np.concatenate([np.asarray(res.results[i]["out"])
                           for i in range(NCORES)], axis=0)
    return outs.astype(np.float32).reshape(B, C, T_).swapaxes(1, 2)


# revision 3
# speedup vs baseline: 2.7085x; 1.2229x over previous
import os
from contextlib import ExitStack

import numpy as np

import jax

jax.config.update("jax_compilation_cache_dir",
                  os.path.expanduser("~/.cache/jax_bass_cache"))
jax.config.update("jax_persistent_cache_min_compile_time_secs", 0.0)
jax.config.update("jax_persistent_cache_min_entry_size_bytes", -1)

import concourse.bass as bass
import concourse.mybir as mybir
from concourse.bass_utils import run_bass_kernel_spmd

F32 = mybir.dt.float32
F16 = mybir.dt.float16
AF = mybir.ActivationFunctionType
OP = mybir.AluOpType

T = 4096
ROWS = 128
NCORES = 8
NBLK = 32           # data blocks of 128 along time
NBLKP = NBLK + 2    # plus one reflect-pad block each side
PADW = NBLKP * 128  # 4352
HW = 2048           # half width (16 blocks)
SIGMAS = (2.5, 4.0, 6.0, 9.0, 14.0)
ZCLAMP = 200.0
NKM = 21            # ident, K0, K1, K0f, 5 sigmas x 3, reflectL, reflectR

LAST_EXEC_NS = None


def _gk(sigma):
    R = max(1, int(4.0 * sigma + 0.5))
    R = min(R, max(1, (T - 1) // 2))
    xs = np.arange(-R, R + 1, dtype=np.float32)
    k = np.exp(np.float32(-0.5) * (xs / np.float32(sigma)) ** 2).astype(np.float32)
    return R, k / (k.sum() + np.float32(1e-12))


def _stationaries():
    """[128, NKM*128] fp16. Layout: 0 identity, 1 K0box, 2 K1box, 3 K0fbox,
    4+3s+g gaussian sigma s shift g. Matmul computes out[u,r] = sum_p M[p,u]X[p,r]."""
    p = np.arange(128)[:, None]
    u = np.arange(128)[None, :]
    mats = np.zeros((NKM, 128, 128), np.float32)
    mats[0] = np.eye(128, dtype=np.float32)
    mats[1] = ((p >= u - 15) & (p <= u)).astype(np.float32)          # K0 causal box
    mats[2] = ((p - 128 >= u - 15) & (p - 128 <= u)).astype(np.float32)  # K1 prev blk
    mats[3] = mats[1].copy()                                          # K0f: + replicate
    for uu in range(15):
        mats[3][0, uu] += 15 - uu
    for si, sig in enumerate(SIGMAS):
        R, k = _gk(sig)
        for g in range(3):
            j = (g - 1) * 128 + p - u + R
            m = (j >= 0) & (j <= 2 * R)
            mats[4 + 3 * si + g][m] = k[np.clip(j, 0, 2 * R)][m]
    mats[19] = (p + u == 128).astype(np.float32)   # left reflect: out[u]=in[128-u]
    mats[20] = (p + u == 126).astype(np.float32)   # right reflect: out[u]=in[126-u]
    km = np.concatenate([mats[i] for i in range(NKM)], axis=1)
    return np.ascontiguousarray(km.astype(np.float16))


_KM = _stationaries()
_CE = (1.0 / np.minimum(np.arange(1, 129, dtype=np.float32), 16.0)).reshape(128, 1)


class _Op:
    __slots__ = ("eng", "fn", "inc", "cnt", "deps", "dma")

    def __init__(self, eng, fn, inc, cnt, deps, dma):
        self.eng, self.fn, self.inc, self.cnt = eng, fn, inc, cnt
        self.deps, self.dma = deps, dma


class Sched:
    """Per-engine in-order queues with explicit cross-engine deps, emitted as
    one semaphore per engine (then_inc after every op, wait_ge before ops with
    unseen dependency counts)."""

    ENGS = ("p", "a", "v", "g", "s")

    def __init__(self):
        self.ops = []
        self.cnt = {e: 0 for e in self.ENGS}

    def add(self, eng, fn, deps=(), dma=False):
        inc = 16 if dma else 1
        self.cnt[eng] += inc
        op = _Op(eng, fn, inc, self.cnt[eng], tuple(d for d in deps if d is not None),
                 dma)
        self.ops.append(op)
        return op

    def emit(self, nc):
        per = {e: [] for e in self.ENGS}
        observed = {e: {o: 0 for o in self.ENGS} for e in self.ENGS}
        for op in self.ops:
            ws = []
            for dep in op.deps:
                if dep.eng == op.eng and not dep.dma:
                    continue  # same-engine program order
                if observed[op.eng][dep.eng] < dep.cnt:
                    ws.append((dep.eng, dep.cnt))
                    observed[op.eng][dep.eng] = dep.cnt
            per[op.eng].append((op, ws))

        with ExitStack() as sctx:
            sems = {e: sctx.enter_context(nc.semaphore(f"sem_{e}"))
                    for e in self.ENGS}
            with nc.Block() as block:
                def replay(engobj, ename):
                    for op, ws in per[ename]:
                        for (o, val) in ws:
                            engobj.wait_ge(sems[o], val)
                        op.fn(engobj).then_inc(sems[ename], op.inc)
                    if ename == "s" and self.cnt["s"]:
                        engobj.wait_ge(sems["s"], self.cnt["s"])

                @block.tensor
                def _(e):
                    replay(nc.tensor, "p")

                @block.scalar
                def _(e):
                    replay(nc.scalar, "a")

                @block.vector
                def _(e):
                    replay(nc.vector, "v")

                @block.gpsimd
                def _(e):
                    replay(nc.gpsimd, "g")

                @block.sync
                def _(e):
                    replay(nc.sync, "s")


def _build(W1, b1, W2, b2, detect_races=True):
    nc = bass.Bass(detect_race_conditions=detect_races)
    xr_d = nc.dram_tensor("xr", [2, NBLK, 128, 64], F16, kind="ExternalInput")
    km_d = nc.dram_tensor("km", [ROWS, NKM * 128], F16, kind="ExternalInput")
    ce_d = nc.dram_tensor("ce", [ROWS, 1], F32, kind="ExternalInput")
    out_d = nc.dram_tensor("out", [ROWS, T], F16, kind="ExternalOutput")

    W1 = np.asarray(W1, np.float64)
    b1 = np.asarray(b1, np.float64)
    w2p = np.asarray(W2, np.float64) / 0.7
    b2p = np.asarray(b2, np.float64) / 0.7

    # Pre-register activation bias consts (activation() converts float biases
    # to const APs, which must be materialized before the Block bodies).
    def reg_const(val):
        key = (F32, float(val))
        if key not in nc.const_aps.aps:
            t = nc.alloc_sbuf_tensor(f"c-{len(nc.const_aps.aps)}", [128, 1], F32)
            nc.gpsimd.memset(t.ap(), float(val))
            nc.const_aps.aps[key] = t.ap()

    for v in [0.0, 1e-6] + [float(x) for x in b1]:
        reg_const(v)
    nc.all_engine_barrier()

    with nc.allow_low_precision("fp16 pipeline by design"), ExitStack() as ctx:
        sb = lambda name, w, dt=F16: ctx.enter_context(  # noqa: E731
            nc.sbuf_tensor(name, [ROWS, w], dt))

        km = sb("km_sb", NKM * 128)
        ce = sb("ce_sb", 1, F32)
        Xt = sb("Xt", PADW)
        xsq = sb("xsq", T)
        z = sb("z", T)
        lv = sb("lv", T)
        Ys = [sb(f"Y{s}", T) for s in range(5)]
        las = [sb(f"la{s}", T) for s in range(5)]
        hb = [sb("hb0", T), sb("hb1", T)]
        sm1 = sb("sm1", HW, F32)
        sm2 = sb("sm2", HW, F32)
        sm3 = sb("sm3", HW, F32)
        acc = sb("acc", T)
        outr = sb("outr", T)

        ident = km[:, 0:128]

        def kmat(i):
            return km[:, i * 128:(i + 1) * 128]

        S = Sched()

        # time-major strided load: xr[b, blk, p, c] -> Xt[p, 128 + blk*128 + b*64 + c]
        d_x = None
        for b in range(2):
            for blk in range(NBLK):
                o = 128 + blk * 128 + b * 64
                d_x = S.add("s", lambda e, b=b, blk=blk, o=o: e.dma_start(
                    Xt[:, o:o + 64], xr_d[b, blk]), dma=True)
        d_k = S.add("s", lambda e: e.dma_start(km[:], km_d[:]), dma=True)
        d_c = S.add("s", lambda e: e.dma_start(ce[:], ce_d[:]), dma=True)

        # ---- phase 1: reflect pad blocks via anti-diagonal matmuls ----
        cps = []
        with ExitStack() as pctx:
            pt = pctx.enter_context(nc.psum_tensor("pt", [128, 128], F32))
            tr = S.add("p", lambda e: e.matmul(
                pt[:], kmat(19), Xt[:, 128:256], start=True, stop=True,
                is_transpose=False), [d_x, d_k])
            cps.append(S.add("a", lambda e: e.activation(
                Xt[:, 0:128], pt[:], AF.Copy), [tr]))
            tr2 = S.add("p", lambda e: e.matmul(
                pt[:], kmat(20), Xt[:, 128 + T - 128:128 + T], start=True,
                stop=True, is_transpose=False), [cps[0]])
            cps.append(S.add("a", lambda e: e.activation(
                Xt[:, 128 + T:PADW], pt[:], AF.Copy), [tr2]))
        xsq_op = S.add("a", lambda e: e.activation(
            xsq[:], Xt[:, 128:128 + T], AF.Square), [d_x])

        # ---- phase 2: causal window sums via PE + stats math ----
        with ExitStack() as pctx:
            ps1 = pctx.enter_context(nc.psum_tensor("ps1", [128, HW], F32))
            ps2 = pctx.enter_context(nc.psum_tensor("ps2", [128, HW], F32))
            ps_readers = {0: [cps[-1]], 1: [cps[-1], xsq_op]}
            zlv_ops = []
            sm_free = []   # ops that must finish before sm1/sm2/sm3 are reused

            def box_mms(ps, k0src, k1src, special, deps):
                """Causal box sums into ps[:, 0:HW], chunked per PSUM bank.
                k0src(lo, hi) / k1src(lo, hi) give moving APs for the chunk;
                special: (kidx, ap) overrides chunk [0:128] with one matmul."""
                ops = []
                lo0 = 0
                if special is not None:
                    kidx, ap = special
                    ops.append(S.add("p", lambda e, kidx=kidx, ap=ap: e.matmul(
                        ps[:, 0:128], kmat(kidx), ap, start=True, stop=True),
                        deps))
                    deps = ()
                    lo0 = 128
                for ci in range(4):
                    lo, hi = max(512 * ci, lo0), 512 * (ci + 1)
                    ops.append(S.add("p", lambda e, lo=lo, hi=hi: e.matmul(
                        ps[:, lo:hi], kmat(1), k0src(lo, hi),
                        start=True, stop=False), deps))
                    deps = ()
                    ops.append(S.add("p", lambda e, lo=lo, hi=hi: e.matmul(
                        ps[:, lo:hi], kmat(2), k1src(lo, hi),
                        start=False, stop=True)))
                return ops

            for h in range(2):
                c0 = h * HW
                if h == 0:
                    mS = box_mms(ps1,
                                 lambda lo, hi: Xt[:, 128 + lo:128 + hi],
                                 lambda lo, hi: Xt[:, lo:hi],
                                 (3, Xt[:, 128:256]), ps_readers[0])
                    mQ = box_mms(ps2,
                                 lambda lo, hi: xsq[:, lo:hi],
                                 lambda lo, hi: xsq[:, lo - 128:hi - 128],
                                 (3, xsq[:, 0:128]), ps_readers[1])
                else:
                    mS = box_mms(ps1,
                                 lambda lo, hi: Xt[:, 128 + HW + lo:128 + HW + hi],
                                 lambda lo, hi: Xt[:, HW + lo:HW + hi],
                                 None, ps_readers[0])
                    mQ = box_mms(ps2,
                                 lambda lo, hi: xsq[:, HW + lo:HW + hi],
                                 lambda lo, hi: xsq[:, HW - 128 + lo:HW - 128 + hi],
                                 None, ps_readers[1])
                # mean / mean2 (PSUM f32 -> SBUF f32, per-position 1/eff)
                if h == 0:
                    am1a = S.add("a", lambda e: e.activation(
                        sm1[:, 0:128], ps1[:, 0:128], AF.Copy,
                        scale=ce[:, 0:1]), [mS[-1], d_c])
                    am1 = S.add("a", lambda e: e.activation(
                        sm1[:, 128:HW], ps1[:, 128:HW], AF.Copy,
                        scale=1.0 / 16.0), [mS[-1]])
                    am2a = S.add("a", lambda e: e.activation(
                        sm2[:, 0:128], ps2[:, 0:128], AF.Copy,
                        scale=ce[:, 0:1]), [mQ[-1], d_c])
                    am2 = S.add("a", lambda e: e.activation(
                        sm2[:, 128:HW], ps2[:, 128:HW], AF.Copy,
                        scale=1.0 / 16.0), [mQ[-1]])
                    mean_ops = [am1a, am1]
                    mean2_ops = [am2a, am2]
                else:
                    am1 = S.add("a", lambda e: e.activation(
                        sm1[:], ps1[:], AF.Copy, scale=1.0 / 16.0),
                        [mS[-1]] + sm_free)
                    am2 = S.add("a", lambda e: e.activation(
                        sm2[:], ps2[:], AF.Copy, scale=1.0 / 16.0),
                        [mQ[-1]] + sm_free)
                    mean_ops = [am1]
                    mean2_ops = [am2]
                ps_readers = {0: mean_ops, 1: mean2_ops}

                v1 = S.add("v", lambda e: e.tensor_mul(sm3[:], sm1[:], sm1[:]),
                           mean_ops)
                v2 = S.add("v", lambda e: e.tensor_sub(sm2[:], sm2[:], sm3[:]),
                           mean2_ops)
                v3 = S.add("v", lambda e: e.tensor_scalar_max(sm2[:], sm2[:], 0.0))
                a3 = S.add("a", lambda e: e.activation(
                    sm3[:], sm2[:], AF.Sqrt, bias=1e-6), [v3])
                a4 = S.add("a", lambda e, c0=c0: e.activation(
                    lv[:, c0:c0 + HW], sm2[:], AF.Ln, bias=1e-6), [v3])
                v4 = S.add("v", lambda e: e.reciprocal(sm3[:], sm3[:]), [a3])
                v5 = S.add("v", lambda e, c0=c0: e.tensor_sub(
                    sm1[:], Xt[:, 128 + c0:128 + c0 + HW], sm1[:]), [a4])
                v6 = S.add("v", lambda e: e.tensor_mul(sm1[:], sm1[:], sm3[:]))
                v7 = S.add("v", lambda e, c0=c0: e.tensor_scalar(
                    z[:, c0:c0 + HW], sm1[:], ZCLAMP, -ZCLAMP, OP.min, OP.max))
                zlv_ops += [v7, a4]
                sm_free = [v7, v6, a4]
                ps_readers = {0: mean_ops, 1: mean2_ops}

            # ---- phase 3: gaussian convs via PE ----
            pgs = [ps1, ps2]
            g_copy = []
            for idx in range(10):
                si, h = divmod(idx, 2)
                c0 = h * HW
                pg = pgs[idx % 2]
                deps = [g_copy[idx - 2]] if idx >= 2 else list(ps_readers[idx])
                last = None
                for g in range(3):
                    for ci in range(4):
                        lo, hi = 512 * ci, 512 * (ci + 1)
                        last = S.add("p", lambda e, si=si, g=g, pg=pg,
                                     s0=c0 + g * 128 + lo, s1=c0 + g * 128 + hi,
                                     lo=lo, hi=hi: e.matmul(
                                         pg[:, lo:hi], kmat(4 + 3 * si + g),
                                         Xt[:, s0:s1],
                                         start=(g == 0), stop=(g == 2)), deps)
                        deps = ()
                g_copy.append(S.add("a", lambda e, si=si, c0=c0, pg=pg:
                                    e.activation(Ys[si][:, c0:c0 + HW], pg[:],
                                                 AF.Copy), [last]))

        # ---- phase 4: gating MLP (elementwise, DVE + ACT) ----
        gels = []
        for j in range(32):
            a = float(W1[j, 0])
            b = float(W1[j, 1])
            cj = float(b1[j])
            h = hb[j % 2]
            hbfree = [gels[j - 2]] if j >= 2 else []
            if a == 0.0 and b == 0.0:
                gel = S.add("a", lambda e, h=h, cj=cj: e.activation(
                    h[:], z[:], AF.Gelu, bias=cj, scale=0.0), zlv_ops + hbfree)
            else:
                if abs(a) >= abs(b):
                    pre = S.add("v", lambda e, h=h, r=b / a: e.scalar_tensor_tensor(
                        h[:], lv[:], r, z[:], OP.mult, OP.add), zlv_ops + hbfree)
                    sc = a
                else:
                    pre = S.add("v", lambda e, h=h, r=a / b: e.scalar_tensor_tensor(
                        h[:], z[:], r, lv[:], OP.mult, OP.add), zlv_ops + hbfree)
                    sc = b
                gel = S.add("a", lambda e, h=h, cj=cj, sc=sc: e.activation(
                    h[:], h[:], AF.Gelu, bias=cj, scale=sc), [pre])
            gels.append(gel)
            for s in range(5):
                w = float(w2p[s, j])
                if j == 0:
                    S.add("v", lambda e, s=s, h=h, w=w, b0=float(b2p[s]):
                          e.tensor_scalar(las[s][:], h[:], w, b0, OP.mult, OP.add),
                          [gel])
                else:
                    S.add("v", lambda e, s=s, h=h, w=w: e.scalar_tensor_tensor(
                        las[s][:], h[:], w, las[s][:], OP.mult, OP.add), [gel])

        # ---- phase 5: softmax + mix ----
        mx = hb[0]
        den = hb[1]
        S.add("v", lambda e: e.tensor_tensor(mx[:], las[0][:], las[1][:], OP.max),
              [gels[-1]])
        for s in (2, 3, 4):
            S.add("v", lambda e, s=s: e.tensor_tensor(mx[:], mx[:], las[s][:],
                                                      OP.max))
        subs = [S.add("v", lambda e, s=s: e.tensor_sub(las[s][:], las[s][:], mx[:]))
                for s in range(5)]
        exps = [S.add("a", lambda e, s=s: e.activation(las[s][:], las[s][:], AF.Exp),
                      [subs[s]]) for s in range(5)]
        S.add("v", lambda e: e.tensor_add(den[:], las[0][:], las[1][:]),
              [exps[0], exps[1]])
        for s in (2, 3, 4):
            S.add("v", lambda e, s=s: e.tensor_add(den[:], den[:], las[s][:]),
                  [exps[s]])
        S.add("v", lambda e: e.reciprocal(den[:], den[:]))
        S.add("v", lambda e: e.tensor_mul(acc[:], las[0][:], Ys[0][:]))
        tmps = [z, lv]
        for s in range(1, 5):
            t = tmps[(s - 1) % 2]
            S.add("v", lambda e, s=s, t=t: e.tensor_mul(t[:], las[s][:], Ys[s][:]))
            S.add("v", lambda e, t=t: e.tensor_add(acc[:], acc[:], t[:]))
        vfin = S.add("v", lambda e: e.tensor_mul(acc[:], acc[:], den[:]))

        # ---- phase 6: transpose back to row-major and store ----
        with ExitStack() as pctx:
            pts = [pctx.enter_context(nc.psum_tensor(f"pu{i}", [128, 128], F16))
                   for i in range(4)]
            ocp = []
            for bidx in range(NBLK):
                deps = [vfin, g_copy[-1]] + ([ocp[bidx - 4]] if bidx >= 4 else [])
                tr = S.add("p", lambda e, b=bidx, pt=pts[bidx % 4]: e.transpose(
                    pt[:], acc[:, b * 128:(b + 1) * 128], ident), deps)
                ocp.append(S.add("a", lambda e, b=bidx, pt=pts[bidx % 4]:
                                 e.activation(outr[:, b * 128:(b + 1) * 128],
                                              pt[:], AF.Copy), [tr]))
        S.add("s", lambda e: e.dma_start(out_d[:], outr[:]), [ocp[-1]], dma=True)

        S.emit(nc)
    return nc


_PROG_CACHE = {}


def _get_program(W1, b1, W2, b2):
    key = (np.asarray(W1, np.float32).tobytes(), np.asarray(b1, np.float32).tobytes(),
           np.asarray(W2, np.float32).tobytes(), np.asarray(b2, np.float32).tobytes())
    prog = _PROG_CACHE.get(key)
    if prog is None:
        prog = _build(W1, b1, W2, b2)
        _PROG_CACHE.clear()
        _PROG_CACHE[key] = prog
    return prog


def kernel(x, W1, b1, W2, b2):
    global LAST_EXEC_NS
    x = np.asarray(x)
    B, T_, C = x.shape
    x16 = x.astype(np.float16).reshape(B // 2, 2, NBLK, 128, C)

    nc = _get_program(W1, b1, W2, b2)
    in_maps = [
        {"xr": x16[i], "km": _KM, "ce": _CE}
        for i in range(NCORES)
    ]
    res = run_bass_kernel_spmd(nc, in_maps, core_ids=list(range(NCORES)),
                               trace=bool(os.environ.get("KBENCH_TRACE")))
    LAST_EXEC_NS = getattr(res, "exec_time_ns", None)
    outs = np.concatenate([np.asarray(res.results[i]["out"])
                           for i in range(NCORES)], axis=0)
    return outs.astype(np.float32).reshape(B, C, T_).swapaxes(1, 2)
